# revision 10
# baseline (speedup 1.0000x reference)
"""Distributed Trainium2 kernel for AsymmetricRoPECrossAttention (v3).

Reference computation (b=2, n_q=2048, n_kv=4096, dim=1024, 16 heads x 64):
    q  = rope(q_x @ Wq);  k = rope(kv_x @ Wk);  v = kv_x @ Wv
    out = softmax(q k^T / sqrt(64)) v @ Wout        (mask is all-ones)

Sharding over 8 cores: batch (2) x head-groups (4 heads each).
Core c: batch bi=c//4, group-rank r=c%4, heads [4r, 4r+4).

v3 design (vs the 863us baseline):
  - Scores as head-PAIRS via PE row-tiling: heads 2hp/2hp+1 live on
    partitions 0-63/64-127 of qr/kr, so two 64-contraction matmuls at
    tile_position (0,0)/(64,0) run concurrently -> 2x score throughput.
  - The 33.5M-element/core exp drain is split: ScalarE runs exact LUT exp
    (scaled by C via bias=ln C) for 22 of every 32 k-tiles; VectorE runs a
    dual-phase Schraudolph for the other 10: P1 = bf16-bits(round(s*A+B)),
    P2 = bits(P1_int + 64), and P1 + 0.707*P2 ~ C*exp(s/8) within +-1%
    (the combination happens inside the PV matmul as two accumulating
    passes against V and 0.707*V, so VectorE pays only 2 cheap ops).
  - PE is kept strictly the bottleneck and the score/PV work is emitted in
    3-k-tile chunks (scores of chunk c, then PV of chunk c-1) so the PE
    stream has no micro-gaps and the HAM clock gate stays at 2.4 GHz (the
    baseline ran all of phase B throttled to 1.2 GHz).
  - Softmax normalization without PE/PSUM: reciprocal rows are partition-
    broadcast by a 0-stride DMA and applied by GpSimd tensor-multiply.
  - Startup DMA ordering: first K-proj matmul no longer waits behind the
    full 16MB initial burst.
"""

import math

import numpy as np
import ml_dtypes

import concourse.bass as bass
import concourse.bacc as bacc
import concourse.mybir as mybir
import concourse.tile as tile
from concourse.bass_utils import run_bass_kernel_spmd

B = 2
NQ = 2048
NKV = 4096
DIM = 1024
HEADS = 16
DH = 64
SCALE = DH ** -0.5
NCORES = 8
GH = 4          # heads per core
GD = GH * DH    # 256 head-dims per core
QS = NQ // 4    # 512 q rows owned per core after the exchange
NQB = NQ // 512
NKB = NKV // 512
NCT = DIM // 128
NKT = NKV // 128

BF16 = mybir.dt.bfloat16
F32 = mybir.dt.float32
I16 = mybir.dt.int16
BF16_NP = ml_dtypes.bfloat16

# Dual-phase Schraudolph constants: I1 = round(s*A16 + B16) is the int16 bit
# pattern of bf16 ~2^(s/8*log2 e); P1 + W2 * bits(I1+64) ~ C_EXP * exp(s/8)
# within +-1%. ScalarE's exact-exp tiles match the C_EXP scale via bias.
A16 = 128.0 * math.log2(math.e) / 8.0
B16 = 16256.0
W2 = 0.707
C_EXP = 2.081186
LNC = math.log(C_EXP)

CH = 3           # k-tiles per PE emission chunk
DVE_KT = (2, 7, 13, 18, 23, 29)   # k-tiles routed to VectorE (6 of 32)


def _rope_tables(seq_len: int):
    """Return (cos, sin_signed) as [128, seq_len] f32, tiled for 2 heads."""
    pos = np.arange(seq_len, dtype=np.float64)[:, None]
    div = np.exp(np.arange(0, DH, 2, dtype=np.float64) * (-math.log(10000.0) / DH))
    freqs = pos * div  # [s, 32]
    emb = np.concatenate([freqs, freqs], axis=1)  # [s, 64]
    cos = np.cos(emb).T.astype(np.float32)  # [64, s]
    sin = np.sin(emb).T.astype(np.float32)
    sin_signed = sin.copy()
    sin_signed[:32] = -sin_signed[:32]
    return np.tile(cos, (2, 1)), np.tile(sin_signed, (2, 1))


def build_nc() -> bass.Bass:
    nc = bacc.Bacc(
        "TRN2", target_bir_lowering=False, debug=False, num_devices=NCORES
    )

    qxT = nc.declare_dram_parameter("q_xT", [DIM, NQ], BF16, isOutput=False)
    kvxT = nc.declare_dram_parameter("kv_xT", [DIM, NKV], BF16, isOutput=False)
    wq_d = nc.declare_dram_parameter("wq", [DIM, GD], BF16, isOutput=False)
    wk_d = nc.declare_dram_parameter("wk", [DIM, GD], BF16, isOutput=False)
    wv_d = nc.declare_dram_parameter("wv", [DIM, GD], BF16, isOutput=False)
    wout_d = nc.declare_dram_parameter("wout", [NCORES, GD, DIM], BF16, isOutput=False)
    cosq_d = nc.declare_dram_parameter("cosq", [128, NQ], BF16, isOutput=False)
    sinq_d = nc.declare_dram_parameter("sinq", [128, NQ], BF16, isOutput=False)
    cosk_d = nc.declare_dram_parameter("cosk", [128, NKV], BF16, isOutput=False)
    sink_d = nc.declare_dram_parameter("sink", [128, NKV], BF16, isOutput=False)
    perm_d = nc.declare_dram_parameter("perm", [128, 128], BF16, isOutput=False)
    out_d = nc.declare_dram_parameter("out", [DIM, QS], F32, isOutput=True)

    a2a_in = [nc.dram_tensor(f"a2a_in{h}", [NCORES, DH, QS], BF16)
              for h in range(GH)]
    a2a_out = [nc.dram_tensor(f"a2a_out{h}", [NCORES, DH, QS], BF16)
               for h in range(GH)]

    with tile.TileContext(nc) as tc:
        with (
            tc.tile_pool(name="wpool", bufs=1) as wpool,
            tc.tile_pool(name="big", bufs=1) as big,
        ):
            # --- resident tiles -----------------------------------------------
            wq_sb = wpool.tile([128, NCT, GD], BF16)
            wk_sb = wpool.tile([128, NCT, GD], BF16)
            wv_sb = wpool.tile([128, NCT, GD], BF16)
            wout_sb = wpool.tile([128, 2 * NCORES, DIM], BF16)

            lnc_sb = wpool.tile([128, 1], F32)      # exp bias ln(C_EXP)
            nc.vector.memset(lnc_sb[:, :], LNC)

            qr_sb = big.tile([128, 2, NQ], BF16)    # rope'd Q^T
            kr_sb = big.tile([128, 2, NKV], BF16)   # rope'd K^T
            v_sb = big.tile([128, NKT, GH, DH + 1], BF16)   # V + ones col
            v2_sb = big.tile([128, NKT, GH, DH + 1], BF16)  # W2 * (V + ones)
            at_sb = big.tile([64, GH, NQ], BF16)    # normalized attention out^T
            obuf = big.tile([65, 2, NQB, 512], F32)  # O^T + sums staging
            srow = big.tile([64, 512], F32)          # sums rows 32*hl + qb
            rcp = big.tile([64, 1, 512], F32)
            rhs_sb = big.tile([128, 2, NCORES, QS], BF16)

            # K-projection critical path first
            for ct in range(NCT):
                nc.gpsimd.dma_start(wk_sb[:, ct, :], wk_d[ct * 128:(ct + 1) * 128, :])
            for ct in range(NCT):
                nc.gpsimd.dma_start(wv_sb[:, ct, :], wv_d[ct * 128:(ct + 1) * 128, :])
            nc.vector.memset(v_sb[:, :, :, DH:DH + 1], 1.0)

            # --- phase A: projections + RoPE ----------------------------------
            with (
                tc.tile_pool(name="ptmp", bufs=4) as ptmp,
                tc.tile_pool(name="ppsum", bufs=3, space="PSUM") as ppsum,
                tc.tile_pool(name="vpsum", bufs=2, space="PSUM") as vpsum,
                tc.tile_pool(name="shpsum", bufs=2, space="PSUM") as shpsum,
            ):
                kt_ctx = tc.tile_pool(name="ktab", bufs=1)
                ktab = kt_ctx.__enter__()
                cosk_sb = ktab.tile([128, NKV], BF16)
                sink_sb = ktab.tile([128, NKV], BF16)
                perm_sb = ptmp.tile([128, 128], BF16, tag="perm", bufs=1)
                nc.gpsimd.dma_start(perm_sb[:, :], perm_d[:, :])
                nc.gpsimd.dma_start(cosk_sb[:, :], cosk_d[:, :])
                nc.gpsimd.dma_start(sink_sb[:, :], sink_d[:, :])

                qxT_r = qxT.ap().rearrange("(c p) n -> p c n", p=128)
                kvxT_r = kvxT.ap().rearrange("(c p) n -> p c n", p=128)

                def rope_nt(dst_col, ps, cos_sb, sin_sb, col0, nt):
                    """dst[:, nt, col0:col0+512] = rope(ps) via PE perm shuffle."""
                    xt16 = ptmp.tile([128, 512], BF16, tag="xt16")
                    nc.vector.tensor_copy(xt16[:, :], ps[:, :])
                    shp = shpsum.tile([128, 512], F32, tag="shp")
                    nc.tensor.matmul(shp[:, :], perm_sb[:, :], xt16[:, :],
                                     start=True, stop=True)
                    cs = cos_sb[:, col0:col0 + 512]
                    sn = sin_sb[:, col0:col0 + 512]
                    tmp = ptmp.tile([128, 512], BF16, tag="tmp")
                    nc.vector.scalar_tensor_tensor(
                        tmp[:, :], xt16[:, :], 1.0, cs,
                        op0=mybir.AluOpType.mult, op1=mybir.AluOpType.mult,
                    )
                    shm = ptmp.tile([128, 512], BF16, tag="shm")
                    nc.vector.scalar_tensor_tensor(
                        shm[:, :], shp[:, :], 1.0, sn,
                        op0=mybir.AluOpType.mult, op1=mybir.AluOpType.mult,
                    )
                    nc.vector.scalar_tensor_tensor(
                        dst_col[:, nt, col0:col0 + 512], tmp[:, :], 0.0, shm[:, :],
                        op0=mybir.AluOpType.add, op1=mybir.AluOpType.add,
                    )

                # K / V projections (+ rope on K) first: phase B waits on these
                xkv_ctx = tc.tile_pool(name="xkv", bufs=3)
                xin = xkv_ctx.__enter__()
                for kb in range(NKB):
                    xt = xin.tile([128, NCT, 512], BF16, tag="xin", name=f"xkv{kb}")
                    nc.gpsimd.dma_start(
                        xt[:, :, :], kvxT_r[:, :, kb * 512:(kb + 1) * 512]
                    )
                    if kb == 2:
                        # deferred: needed from the Q-projection onwards
                        for ct in range(NCT):
                            nc.gpsimd.dma_start(
                                wq_sb[:, ct, :], wq_d[ct * 128:(ct + 1) * 128, :])
                    for nt in range(2):
                        ps = ppsum.tile([128, 512], F32, tag="ppsum", name=f"kp{nt}")
                        for ct in range(NCT):
                            nc.tensor.matmul(
                                ps[:, :],
                                wk_sb[:, ct, nt * 128:(nt + 1) * 128],
                                xt[:, ct, :],
                                start=(ct == 0), stop=(ct == NCT - 1),
                            )
                        rope_nt(kr_sb, ps, cosk_sb, sink_sb, kb * 512, nt)
                    for sub in range(4):
                        vps = vpsum.tile([128, GD], F32, tag="vpsum", name=f"vp{sub}")
                        for ct in range(NCT):
                            # column-packed pair: both halves share the wv stream
                            for half in range(2):
                                nc.tensor.matmul(
                                    vps[half * 64:(half + 1) * 64, :],
                                    xt[:, ct, sub * 128 + half * 64:
                                       sub * 128 + (half + 1) * 64],
                                    wv_sb[:, ct, :],
                                    start=(ct == 0), stop=(ct == NCT - 1),
                                    tile_position=(0, half * 64),
                                )
                        kt = kb * 4 + sub
                        nc.vector.tensor_copy(
                            v_sb[:, kt, :, 0:DH],
                            vps[:, :].rearrange("p (h d) -> p h d", h=GH),
                        )
                        nc.vector.tensor_scalar_mul(
                            v2_sb[:, kt, :, :], v_sb[:, kt, :, :], W2,
                        )
                xkv_ctx.__exit__(None, None, None)
                kt_ctx.__exit__(None, None, None)

                # Q projection + rope
                q_ctx = tc.tile_pool(name="qtab", bufs=1)
                qtab = q_ctx.__enter__()
                cosq_sb = qtab.tile([128, NQ], BF16)
                sinq_sb = qtab.tile([128, NQ], BF16)
                nc.gpsimd.dma_start(cosq_sb[:, :], cosq_d[:, :])
                nc.gpsimd.dma_start(sinq_sb[:, :], sinq_d[:, :])
                xq_ctx = tc.tile_pool(name="xq", bufs=3)
                xin = xq_ctx.__enter__()
                for qb in range(NQB):
                    xt = xin.tile([128, NCT, 512], BF16, tag="xq", name=f"xq{qb}")
                    nc.gpsimd.dma_start(
                        xt[:, :, :], qxT_r[:, :, qb * 512:(qb + 1) * 512]
                    )
                    for nt in range(2):
                        ps = ppsum.tile([128, 512], F32, tag="ppsum", name=f"qp{nt}")
                        for ct in range(NCT):
                            nc.tensor.matmul(
                                ps[:, :],
                                wq_sb[:, ct, nt * 128:(nt + 1) * 128],
                                xt[:, ct, :],
                                start=(ct == 0), stop=(ct == NCT - 1),
                            )
                        rope_nt(qr_sb, ps, cosq_sb, sinq_sb, qb * 512, nt)
                xq_ctx.__exit__(None, None, None)
                q_ctx.__exit__(None, None, None)

                # deferred: only needed by phase C
                for i in range(NCORES):
                    for sub in range(2):
                        nc.gpsimd.dma_start(
                            wout_sb[:, 2 * i + sub, :],
                            wout_d[i, sub * 128:(sub + 1) * 128, :],
                        )

            # --- phase B: attention, head-pair streams ------------------------
            with (
                tc.tile_pool(name="ptp", bufs=3) as ptp,
                tc.tile_pool(name="p1p", bufs=2) as p1p,
                tc.tile_pool(name="p2p", bufs=2) as p2p,
                tc.tile_pool(name="bcp", bufs=2) as bcp,
                tc.tile_pool(name="stps", bufs=3, space="PSUM") as stps,
                tc.tile_pool(name="otps", bufs=1, space="PSUM") as otps,
            ):
                for hp in range(2):
                    hA, hB = 2 * hp, 2 * hp + 1
                    for qb in range(NQB):
                        qc = qb * 512
                        ot_A = otps.tile([65, 512], F32, tag="otA")
                        ot_B = otps.tile([65, 512], F32, tag="otB")

                        def emit_scores_exp(kt):
                            kc = kt * 128
                            st = stps.tile([128, 2, 512], F32, tag="st",
                                           name=f"st{kt}")
                            nc.tensor.matmul(
                                st[:, 0, :],
                                kr_sb[0:64, hp, kc:kc + 128],
                                qr_sb[0:64, hp, qc:qc + 512],
                                start=True, stop=True,
                            )
                            nc.tensor.matmul(
                                st[:, 1, :],
                                kr_sb[64:128, hp, kc:kc + 128],
                                qr_sb[64:128, hp, qc:qc + 512],
                                start=True, stop=True,
                            )
                            if kt in DVE_KT:
                                p1 = p1p.tile([128, 2, 512], I16, tag="p1",
                                              name=f"p1_{kt}")
                                p2 = p2p.tile([128, 2, 512], I16, tag="p2",
                                              name=f"p2_{kt}")
                                nc.vector.tensor_scalar(
                                    p1[:, :, :], st[:, :, :], A16, B16,
                                    op0=mybir.AluOpType.mult,
                                    op1=mybir.AluOpType.add,
                                )
                                nc.vector.tensor_scalar(
                                    p2[:, :, :], p1[:, :, :], 64, None,
                                    op0=mybir.AluOpType.add,
                                )
                                return (p1, p2)
                            pt = ptp.tile([128, 2, 512], BF16, tag="pt",
                                          name=f"pt{kt}")
                            nc.scalar.activation(
                                pt[:, :, :], st[:, :, :],
                                mybir.ActivationFunctionType.Exp,
                                bias=lnc_sb[:, :], scale=SCALE,
                            )
                            return (pt,)

                        def emit_pv(kt, prod):
                            first, last = kt == 0, kt == NKT - 1
                            if len(prod) == 1:
                                for h_l, ot in ((0, ot_A), (1, ot_B)):
                                    nc.tensor.matmul(
                                        ot[:, :],
                                        v_sb[:, kt, 2 * hp + h_l, :],
                                        prod[0][:, h_l, :],
                                        start=first, stop=last and h_l == 1,
                                    )
                            else:
                                p1, p2 = prod
                                for h_l, ot in ((0, ot_A), (1, ot_B)):
                                    nc.tensor.matmul(
                                        ot[:, :],
                                        v_sb[:, kt, 2 * hp + h_l, :],
                                        p1[:, h_l, :].bitcast(BF16),
                                        start=first, stop=False,
                                    )
                                    nc.tensor.matmul(
                                        ot[:, :],
                                        v2_sb[:, kt, 2 * hp + h_l, :],
                                        p2[:, h_l, :].bitcast(BF16),
                                        start=False, stop=last and h_l == 1,
                                    )

                        # chunked emission: scores of chunk c, PV of chunk c-1
                        prev = []
                        for c0 in range(0, NKT, CH):
                            cur = [(kt, emit_scores_exp(kt))
                                   for kt in range(c0, min(c0 + CH, NKT))]
                            for kt, prod in prev:
                                emit_pv(kt, prod)
                            prev = cur
                        for kt, prod in prev:
                            emit_pv(kt, prod)

                        nc.scalar.copy(obuf[:, 0, qb, :], ot_A[:, :])
                        nc.vector.tensor_copy(obuf[:, 1, qb, :], ot_B[:, :])

                    # normalization: h-even chain on VectorE, h-odd on GpSimd
                    # (parallel queues), reciprocal via fast Newton approx
                    for hl in range(2):
                        h = 2 * hp + hl
                        r0 = 32 * hl
                        dmae = nc.gpsimd
                        mule = nc.gpsimd
                        dmae.dma_start(srow[r0:r0 + NQB, :],
                                       obuf[64:65, hl, :, :])
                        nc.vector.reciprocal(
                            rcp[r0:r0 + NQB, 0, :], srow[r0:r0 + NQB, :])
                        for qb in range(NQB):
                            r = r0 + qb
                            bc = bcp.tile([64, 512], F32, tag=f"bc{hl}",
                                          name=f"bc{h}_{qb}")
                            dmae.dma_start(
                                bc[:, :],
                                rcp[r:r + 1, 0:1, :].to_broadcast([1, 64, 512]),
                            )
                            mule.tensor_tensor(
                                at_sb[:, h, qb * 512:(qb + 1) * 512],
                                obuf[0:DH, hl, qb, :], bc[:, :],
                                op=mybir.AluOpType.mult,
                            )
                        for j in range(NCORES):
                            dmae.dma_start(
                                a2a_in[h][j, :, :],
                                at_sb[:, h, (j % 4) * QS:(j % 4 + 1) * QS],
                            )
                        nc.gpsimd.collective_compute(
                            "AllToAll",
                            mybir.AluOpType.bypass,
                            replica_groups=[list(range(NCORES))],
                            ins=[a2a_in[h].ap().opt()],
                            outs=[a2a_out[h].ap().opt()],
                        )
                        po = 64 * hl
                        for i in range(NCORES):
                            dmae.dma_start(
                                rhs_sb[po:po + DH, hp, i, :], a2a_out[h][i, :, :]
                            )

            # --- phase C: output projection -------------------------------
            with (
                tc.tile_pool(name="cpsum", bufs=1, space="PSUM") as cpsum,
                tc.tile_pool(name="osb", bufs=4) as osb,
            ):
                cps = [cpsum.tile([128, 512], F32, tag=f"cp{et}",
                                  name=f"cp{et}") for et in range(8)]
                for sub in range(2):
                    for et in range(8):
                        for i in range(NCORES):
                            nc.tensor.matmul(
                                cps[et][:, :],
                                wout_sb[:, 2 * i + sub,
                                        et * 128:(et + 1) * 128],
                                rhs_sb[:, sub, i, :],
                                start=(sub == 0 and i == 0),
                                stop=(sub == 1 and i == NCORES - 1),
                            )
                for et in range(8):
                    ob = osb.tile([128, 512], F32, tag="osb")
                    nc.vector.tensor_copy(ob[:, :], cps[et][:, :])
                    nc.gpsimd.dma_start(out_d[et * 128:(et + 1) * 128, :],
                                        ob[:, :])

    nc.compile()
    return nc


_NC_CACHE = None


def _get_nc():
    global _NC_CACHE
    if _NC_CACHE is None:
        _NC_CACHE = build_nc()
    return _NC_CACHE


def kernel(q_x, kv_x, mask, Wq, Wkv, Wout, **_ignored):
    del mask  # all-ones by construction
    q_x = np.asarray(q_x, dtype=np.float32)
    kv_x = np.asarray(kv_x, dtype=np.float32)
    Wq = np.asarray(Wq, dtype=np.float32)
    Wkv = np.asarray(Wkv, dtype=np.float32)
    Wout = np.asarray(Wout, dtype=np.float32)

    cosq, sinq = _rope_tables(NQ)
    cosk, sink = _rope_tables(NKV)
    cosq = cosq.astype(BF16_NP)
    sinq = sinq.astype(BF16_NP)
    cosk = cosk.astype(BF16_NP)
    sink = sink.astype(BF16_NP)

    # rotate-half permutation: perm[k, d]=1 iff d=(k+32)%64 within each 64-block
    perm_np = np.zeros((128, 128), dtype=BF16_NP)
    for k in range(128):
        blk = (k // 64) * 64
        perm_np[k, blk + ((k - blk) + 32) % 64] = 1.0

    qxT = {b: np.ascontiguousarray(q_x[b].T).astype(BF16_NP) for b in range(B)}
    kvxT = {b: np.ascontiguousarray(kv_x[b].T).astype(BF16_NP) for b in range(B)}

    in_maps = []
    for c in range(NCORES):
        bi, r = c // 4, c % 4
        sl = slice(r * GD, (r + 1) * GD)
        wq_c = np.ascontiguousarray(Wq[:, sl]).astype(BF16_NP)
        wk_c = np.ascontiguousarray(Wkv[:, sl]).astype(BF16_NP)
        wv_c = np.ascontiguousarray(Wkv[:, DIM:][:, sl]).astype(BF16_NP)
        # wout shard: slot i holds Wout rows for core i's head block, zeroed
        # when core i belongs to the other batch (kills cross-batch A2A data)
        wout_c = np.zeros((NCORES, GD, DIM), dtype=BF16_NP)
        for i in range(NCORES):
            if i // 4 == bi:
                ri = i % 4
                wout_c[i] = Wout[ri * GD:(ri + 1) * GD, :].astype(BF16_NP)
        in_maps.append({
            "q_xT": qxT[bi],
            "kv_xT": kvxT[bi],
            "wq": wq_c,
            "wk": wk_c,
            "wv": wv_c,
            "wout": wout_c,
            "cosq": cosq,
            "sinq": sinq,
            "cosk": cosk,
            "sink": sink,
            "perm": perm_np,
        })

    nc = _get_nc()
    res = run_bass_kernel_spmd(nc, in_maps, core_ids=list(range(NCORES)))
    results = res.results if hasattr(res, "results") else res

    out = np.empty((B, NQ, DIM), dtype=np.float32)
    for c in range(NCORES):
        bi, r = c // 4, c % 4
        out_c = np.asarray(results[c]["out"], dtype=np.float32)  # [DIM, QS]
        out[bi, r * QS:(r + 1) * QS, :] = out_c.T
    return out


if __name__ == "__main__":
    rng = np.random.default_rng(0)
    inputs = {
        "q_x": rng.standard_normal((B, NQ, DIM), dtype=np.float32),
        "kv_x": rng.standard_normal((B, NKV, DIM), dtype=np.float32),
        "mask": np.ones((B, NKV), dtype=bool),
        "Wq": rng.standard_normal((DIM, DIM), dtype=np.float32) * 0.03,
        "Wkv": rng.standard_normal((DIM, 2 * DIM), dtype=np.float32) * 0.03,
        "Wout": rng.standard_normal((DIM, DIM), dtype=np.float32) * 0.03,
    }
    o = kernel(**inputs)
    print("kernel output", o.shape, o.dtype)


# revision 12
# speedup vs baseline: 1.0026x; 1.0026x over previous
"""Distributed Trainium2 kernel for AsymmetricRoPECrossAttention (v3).

Reference computation (b=2, n_q=2048, n_kv=4096, dim=1024, 16 heads x 64):
    q  = rope(q_x @ Wq);  k = rope(kv_x @ Wk);  v = kv_x @ Wv
    out = softmax(q k^T / sqrt(64)) v @ Wout        (mask is all-ones)

Sharding over 8 cores: batch (2) x head-groups (4 heads each).
Core c: batch bi=c//4, group-rank r=c%4, heads [4r, 4r+4).

v3 design (vs the 863us baseline):
  - Scores as head-PAIRS via PE row-tiling: heads 2hp/2hp+1 live on
    partitions 0-63/64-127 of qr/kr, so two 64-contraction matmuls at
    tile_position (0,0)/(64,0) run concurrently -> 2x score throughput.
  - The 33.5M-element/core exp drain is split: ScalarE runs exact LUT exp
    (scaled by C via bias=ln C) for 22 of every 32 k-tiles; VectorE runs a
    dual-phase Schraudolph for the other 10: P1 = bf16-bits(round(s*A+B)),
    P2 = bits(P1_int + 64), and P1 + 0.707*P2 ~ C*exp(s/8) within +-1%
    (the combination happens inside the PV matmul as two accumulating
    passes against V and 0.707*V, so VectorE pays only 2 cheap ops).
  - PE is kept strictly the bottleneck and the score/PV work is emitted in
    3-k-tile chunks (scores of chunk c, then PV of chunk c-1) so the PE
    stream has no micro-gaps and the HAM clock gate stays at 2.4 GHz (the
    baseline ran all of phase B throttled to 1.2 GHz).
  - Softmax normalization without PE/PSUM: reciprocal rows are partition-
    broadcast by a 0-stride DMA and applied by GpSimd tensor-multiply.
  - Startup DMA ordering: first K-proj matmul no longer waits behind the
    full 16MB initial burst.
"""

import math

import numpy as np
import ml_dtypes

import concourse.bass as bass
import concourse.bacc as bacc
import concourse.mybir as mybir
import concourse.tile as tile
from concourse.bass_utils import run_bass_kernel_spmd

B = 2
NQ = 2048
NKV = 4096
DIM = 1024
HEADS = 16
DH = 64
SCALE = DH ** -0.5
NCORES = 8
GH = 4          # heads per core
GD = GH * DH    # 256 head-dims per core
QS = NQ // 4    # 512 q rows owned per core after the exchange
NQB = NQ // 512
NKB = NKV // 512
NCT = DIM // 128
NKT = NKV // 128

BF16 = mybir.dt.bfloat16
F32 = mybir.dt.float32
I16 = mybir.dt.int16
BF16_NP = ml_dtypes.bfloat16

# Dual-phase Schraudolph constants: I1 = round(s*A16 + B16) is the int16 bit
# pattern of bf16 ~2^(s/8*log2 e); P1 + W2 * bits(I1+64) ~ C_EXP * exp(s/8)
# within +-1%. ScalarE's exact-exp tiles match the C_EXP scale via bias.
A16 = 128.0 * math.log2(math.e) / 8.0
B16 = 16256.0
W2 = 0.707
C_EXP = 2.081186
LNC = math.log(C_EXP)

CH = 3           # k-tiles per PE emission chunk
DVE_KT = (2, 7, 13, 18, 23, 29)   # k-tiles routed to VectorE (6 of 32)


def _rope_tables(seq_len: int):
    """Return (cos, sin_signed) as [128, seq_len] f32, tiled for 2 heads."""
    pos = np.arange(seq_len, dtype=np.float64)[:, None]
    div = np.exp(np.arange(0, DH, 2, dtype=np.float64) * (-math.log(10000.0) / DH))
    freqs = pos * div  # [s, 32]
    emb = np.concatenate([freqs, freqs], axis=1)  # [s, 64]
    cos = np.cos(emb).T.astype(np.float32)  # [64, s]
    sin = np.sin(emb).T.astype(np.float32)
    sin_signed = sin.copy()
    sin_signed[:32] = -sin_signed[:32]
    return np.tile(cos, (2, 1)), np.tile(sin_signed, (2, 1))


def build_nc() -> bass.Bass:
    nc = bacc.Bacc(
        "TRN2", target_bir_lowering=False, debug=False, num_devices=NCORES
    )

    qxT = nc.declare_dram_parameter("q_xT", [DIM, NQ], BF16, isOutput=False)
    kvxT = nc.declare_dram_parameter("kv_xT", [DIM, NKV], BF16, isOutput=False)
    wq_d = nc.declare_dram_parameter("wq", [DIM, GD], BF16, isOutput=False)
    wk_d = nc.declare_dram_parameter("wk", [DIM, GD], BF16, isOutput=False)
    wv_d = nc.declare_dram_parameter("wv", [DIM, GD], BF16, isOutput=False)
    wout_d = nc.declare_dram_parameter("wout", [NCORES, GD, DIM], BF16, isOutput=False)
    cosq_d = nc.declare_dram_parameter("cosq", [128, NQ], BF16, isOutput=False)
    sinq_d = nc.declare_dram_parameter("sinq", [128, NQ], BF16, isOutput=False)
    cosk_d = nc.declare_dram_parameter("cosk", [128, NKV], BF16, isOutput=False)
    sink_d = nc.declare_dram_parameter("sink", [128, NKV], BF16, isOutput=False)
    perm_d = nc.declare_dram_parameter("perm", [128, 128], BF16, isOutput=False)
    out_d = nc.declare_dram_parameter("out", [DIM, QS], F32, isOutput=True)

    a2a_in = [nc.dram_tensor(f"a2a_in{h}", [NCORES, DH, QS], BF16)
              for h in range(GH)]
    a2a_out = [nc.dram_tensor(f"a2a_out{h}", [NCORES, DH, QS], BF16)
               for h in range(GH)]

    with tile.TileContext(nc) as tc:
        with (
            tc.tile_pool(name="wpool", bufs=1) as wpool,
            tc.tile_pool(name="big", bufs=1) as big,
        ):
            # --- resident tiles -----------------------------------------------
            wq_sb = wpool.tile([128, NCT, GD], BF16)
            wk_sb = wpool.tile([128, NCT, GD], BF16)
            wv_sb = wpool.tile([128, NCT, GD], BF16)
            wout_sb = wpool.tile([128, 2 * NCORES, DIM], BF16)

            lnc_sb = wpool.tile([128, 1], F32)      # exp bias ln(C_EXP)
            nc.vector.memset(lnc_sb[:, :], LNC)

            qr_sb = big.tile([128, 2, NQ], BF16)    # rope'd Q^T
            kr_sb = big.tile([128, 2, NKV], BF16)   # rope'd K^T
            v_sb = big.tile([128, NKT, GH, DH + 1], BF16)   # V + ones col
            v2_sb = big.tile([128, NKT, GH, DH + 1], BF16)  # W2 * (V + ones)
            at_sb = big.tile([64, GH, NQ], BF16)    # normalized attention out^T
            obuf = big.tile([65, 2, NQB, 512], F32)  # O^T + sums staging
            rhs_sb = big.tile([128, 2, NCORES, QS], BF16)

            # K-projection critical path first
            for ct in range(NCT):
                nc.gpsimd.dma_start(wk_sb[:, ct, :], wk_d[ct * 128:(ct + 1) * 128, :])
            for ct in range(NCT):
                nc.gpsimd.dma_start(wv_sb[:, ct, :], wv_d[ct * 128:(ct + 1) * 128, :])
            nc.vector.memset(v_sb[:, :, :, DH:DH + 1], 1.0)

            # --- phase A: projections + RoPE ----------------------------------
            with (
                tc.tile_pool(name="ptmp", bufs=4) as ptmp,
                tc.tile_pool(name="ppsum", bufs=3, space="PSUM") as ppsum,
                tc.tile_pool(name="vpsum", bufs=2, space="PSUM") as vpsum,
                tc.tile_pool(name="shpsum", bufs=2, space="PSUM") as shpsum,
            ):
                kt_ctx = tc.tile_pool(name="ktab", bufs=1)
                ktab = kt_ctx.__enter__()
                cosk_sb = ktab.tile([128, NKV], BF16)
                sink_sb = ktab.tile([128, NKV], BF16)
                perm_sb = ptmp.tile([128, 128], BF16, tag="perm", bufs=1)
                nc.gpsimd.dma_start(perm_sb[:, :], perm_d[:, :])
                nc.gpsimd.dma_start(cosk_sb[:, :], cosk_d[:, :])
                nc.gpsimd.dma_start(sink_sb[:, :], sink_d[:, :])

                qxT_r = qxT.ap().rearrange("(c p) n -> p c n", p=128)
                kvxT_r = kvxT.ap().rearrange("(c p) n -> p c n", p=128)

                def rope_nt(dst_col, ps, cos_sb, sin_sb, col0, nt):
                    """dst[:, nt, col0:col0+512] = rope(ps) via PE perm shuffle."""
                    xt16 = ptmp.tile([128, 512], BF16, tag="xt16")
                    nc.vector.tensor_copy(xt16[:, :], ps[:, :])
                    shp = shpsum.tile([128, 512], F32, tag="shp")
                    nc.tensor.matmul(shp[:, :], perm_sb[:, :], xt16[:, :],
                                     start=True, stop=True)
                    cs = cos_sb[:, col0:col0 + 512]
                    sn = sin_sb[:, col0:col0 + 512]
                    tmp = ptmp.tile([128, 512], BF16, tag="tmp")
                    nc.vector.scalar_tensor_tensor(
                        tmp[:, :], xt16[:, :], 1.0, cs,
                        op0=mybir.AluOpType.mult, op1=mybir.AluOpType.mult,
                    )
                    shm = ptmp.tile([128, 512], BF16, tag="shm")
                    nc.vector.scalar_tensor_tensor(
                        shm[:, :], shp[:, :], 1.0, sn,
                        op0=mybir.AluOpType.mult, op1=mybir.AluOpType.mult,
                    )
                    nc.vector.scalar_tensor_tensor(
                        dst_col[:, nt, col0:col0 + 512], tmp[:, :], 0.0, shm[:, :],
                        op0=mybir.AluOpType.add, op1=mybir.AluOpType.add,
                    )

                # K / V projections (+ rope on K) first: phase B waits on these
                xkv_ctx = tc.tile_pool(name="xkv", bufs=3)
                xin = xkv_ctx.__enter__()
                for kb in range(NKB):
                    xt = xin.tile([128, NCT, 512], BF16, tag="xin", name=f"xkv{kb}")
                    nc.gpsimd.dma_start(
                        xt[:, :, :], kvxT_r[:, :, kb * 512:(kb + 1) * 512]
                    )
                    if kb == 2:
                        # deferred: needed from the Q-projection onwards
                        for ct in range(NCT):
                            nc.gpsimd.dma_start(
                                wq_sb[:, ct, :], wq_d[ct * 128:(ct + 1) * 128, :])
                    for nt in range(2):
                        ps = ppsum.tile([128, 512], F32, tag="ppsum", name=f"kp{nt}")
                        for ct in range(NCT):
                            nc.tensor.matmul(
                                ps[:, :],
                                wk_sb[:, ct, nt * 128:(nt + 1) * 128],
                                xt[:, ct, :],
                                start=(ct == 0), stop=(ct == NCT - 1),
                            )
                        rope_nt(kr_sb, ps, cosk_sb, sink_sb, kb * 512, nt)
                    for sub in range(4):
                        vps = vpsum.tile([128, GD], F32, tag="vpsum", name=f"vp{sub}")
                        for ct in range(NCT):
                            # column-packed pair: both halves share the wv stream
                            for half in range(2):
                                nc.tensor.matmul(
                                    vps[half * 64:(half + 1) * 64, :],
                                    xt[:, ct, sub * 128 + half * 64:
                                       sub * 128 + (half + 1) * 64],
                                    wv_sb[:, ct, :],
                                    start=(ct == 0), stop=(ct == NCT - 1),
                                    tile_position=(0, half * 64),
                                )
                        kt = kb * 4 + sub
                        nc.vector.tensor_copy(
                            v_sb[:, kt, :, 0:DH],
                            vps[:, :].rearrange("p (h d) -> p h d", h=GH),
                        )
                        nc.vector.tensor_scalar_mul(
                            v2_sb[:, kt, :, :], v_sb[:, kt, :, :], W2,
                        )
                xkv_ctx.__exit__(None, None, None)
                kt_ctx.__exit__(None, None, None)

                # Q projection + rope
                q_ctx = tc.tile_pool(name="qtab", bufs=1)
                qtab = q_ctx.__enter__()
                cosq_sb = qtab.tile([128, NQ], BF16)
                sinq_sb = qtab.tile([128, NQ], BF16)
                nc.gpsimd.dma_start(cosq_sb[:, :], cosq_d[:, :])
                nc.gpsimd.dma_start(sinq_sb[:, :], sinq_d[:, :])
                xq_ctx = tc.tile_pool(name="xq", bufs=3)
                xin = xq_ctx.__enter__()
                for qb in range(NQB):
                    xt = xin.tile([128, NCT, 512], BF16, tag="xq", name=f"xq{qb}")
                    nc.gpsimd.dma_start(
                        xt[:, :, :], qxT_r[:, :, qb * 512:(qb + 1) * 512]
                    )
                    for nt in range(2):
                        ps = ppsum.tile([128, 512], F32, tag="ppsum", name=f"qp{nt}")
                        for ct in range(NCT):
                            nc.tensor.matmul(
                                ps[:, :],
                                wq_sb[:, ct, nt * 128:(nt + 1) * 128],
                                xt[:, ct, :],
                                start=(ct == 0), stop=(ct == NCT - 1),
                            )
                        rope_nt(qr_sb, ps, cosq_sb, sinq_sb, qb * 512, nt)
                xq_ctx.__exit__(None, None, None)
                q_ctx.__exit__(None, None, None)

                # deferred: only needed by phase C
                for i in range(NCORES):
                    for sub in range(2):
                        nc.gpsimd.dma_start(
                            wout_sb[:, 2 * i + sub, :],
                            wout_d[i, sub * 128:(sub + 1) * 128, :],
                        )

            # --- phase B: attention, head-pair streams ------------------------
            with (
                tc.tile_pool(name="ptp", bufs=3) as ptp,
                tc.tile_pool(name="p1p", bufs=2) as p1p,
                tc.tile_pool(name="p2p", bufs=2) as p2p,
                tc.tile_pool(name="bcp", bufs=2) as bcp,
                tc.tile_pool(name="stps", bufs=3, space="PSUM") as stps,
                tc.tile_pool(name="otps", bufs=1, space="PSUM") as otps,
            ):
                for hp in range(2):
                    hA, hB = 2 * hp, 2 * hp + 1
                    for qb in range(NQB):
                        qc = qb * 512
                        ot_A = otps.tile([65, 512], F32, tag="otA")
                        ot_B = otps.tile([65, 512], F32, tag="otB")

                        def emit_scores_exp(kt):
                            kc = kt * 128
                            st = stps.tile([128, 2, 512], F32, tag="st",
                                           name=f"st{kt}")
                            nc.tensor.matmul(
                                st[:, 0, :],
                                kr_sb[0:64, hp, kc:kc + 128],
                                qr_sb[0:64, hp, qc:qc + 512],
                                start=True, stop=True,
                            )
                            nc.tensor.matmul(
                                st[:, 1, :],
                                kr_sb[64:128, hp, kc:kc + 128],
                                qr_sb[64:128, hp, qc:qc + 512],
                                start=True, stop=True,
                            )
                            if kt in DVE_KT:
                                p1 = p1p.tile([128, 2, 512], I16, tag="p1",
                                              name=f"p1_{kt}")
                                p2 = p2p.tile([128, 2, 512], I16, tag="p2",
                                              name=f"p2_{kt}")
                                nc.vector.tensor_scalar(
                                    p1[:, :, :], st[:, :, :], A16, B16,
                                    op0=mybir.AluOpType.mult,
                                    op1=mybir.AluOpType.add,
                                )
                                nc.vector.tensor_scalar(
                                    p2[:, :, :], p1[:, :, :], 64, None,
                                    op0=mybir.AluOpType.add,
                                )
                                return (p1, p2)
                            pt = ptp.tile([128, 2, 512], BF16, tag="pt",
                                          name=f"pt{kt}")
                            nc.scalar.activation(
                                pt[:, :, :], st[:, :, :],
                                mybir.ActivationFunctionType.Exp,
                                bias=lnc_sb[:, :], scale=SCALE,
                            )
                            return (pt,)

                        def emit_pv(kt, prod):
                            first, last = kt == 0, kt == NKT - 1
                            if len(prod) == 1:
                                for h_l, ot in ((0, ot_A), (1, ot_B)):
                                    nc.tensor.matmul(
                                        ot[:, :],
                                        v_sb[:, kt, 2 * hp + h_l, :],
                                        prod[0][:, h_l, :],
                                        start=first, stop=last and h_l == 1,
                                    )
                            else:
                                p1, p2 = prod
                                for h_l, ot in ((0, ot_A), (1, ot_B)):
                                    nc.tensor.matmul(
                                        ot[:, :],
                                        v_sb[:, kt, 2 * hp + h_l, :],
                                        p1[:, h_l, :].bitcast(BF16),
                                        start=first, stop=False,
                                    )
                                    nc.tensor.matmul(
                                        ot[:, :],
                                        v2_sb[:, kt, 2 * hp + h_l, :],
                                        p2[:, h_l, :].bitcast(BF16),
                                        start=False, stop=last and h_l == 1,
                                    )

                        # chunked emission: scores of chunk c, PV of chunk c-1
                        prev = []
                        for c0 in range(0, NKT, CH):
                            cur = [(kt, emit_scores_exp(kt))
                                   for kt in range(c0, min(c0 + CH, NKT))]
                            for kt, prod in prev:
                                emit_pv(kt, prod)
                            prev = cur
                        for kt, prod in prev:
                            emit_pv(kt, prod)

                        nc.scalar.copy(obuf[:, 0, qb, :], ot_A[:, :])
                        nc.vector.tensor_copy(obuf[:, 1, qb, :], ot_B[:, :])

                        # eager per-q-block normalization: sums for this block
                        # are final now, so 1/s and the broadcast-multiply run
                        # during the next stream instead of serializing the tail
                        for hl in range(2):
                            h = 2 * hp + hl
                            nc.vector.reciprocal(obuf[64:65, hl, qb, :],
                                                 obuf[64:65, hl, qb, :])
                            bc = bcp.tile([64, 512], F32, tag=f"bc{hl}",
                                          name=f"bc{h}_{qb}")
                            nc.gpsimd.dma_start(
                                bc[:, :],
                                obuf[64:65, hl:hl + 1, qb,
                                     :].to_broadcast([1, 64, 512]),
                            )
                            mule = nc.vector if hl == 0 else nc.gpsimd
                            mule.tensor_tensor(
                                at_sb[:, h, qb * 512:(qb + 1) * 512],
                                obuf[0:DH, hl, qb, :], bc[:, :],
                                op=mybir.AluOpType.mult,
                            )

                    # per-head AllToAll
                    for hl in range(2):
                        h = 2 * hp + hl
                        for j in range(NCORES):
                            nc.gpsimd.dma_start(
                                a2a_in[h][j, :, :],
                                at_sb[:, h, (j % 4) * QS:(j % 4 + 1) * QS],
                            )
                        nc.gpsimd.collective_compute(
                            "AllToAll",
                            mybir.AluOpType.bypass,
                            replica_groups=[list(range(NCORES))],
                            ins=[a2a_in[h].ap().opt()],
                            outs=[a2a_out[h].ap().opt()],
                        )
                        po = 64 * hl
                        for i in range(NCORES):
                            nc.gpsimd.dma_start(
                                rhs_sb[po:po + DH, hp, i, :], a2a_out[h][i, :, :]
                            )

            # --- phase C: output projection -------------------------------
            with (
                tc.tile_pool(name="cpsum", bufs=1, space="PSUM") as cpsum,
                tc.tile_pool(name="osb", bufs=4) as osb,
            ):
                cps = [cpsum.tile([128, 512], F32, tag=f"cp{et}",
                                  name=f"cp{et}") for et in range(8)]
                for sub in range(2):
                    for et in range(8):
                        for i in range(NCORES):
                            nc.tensor.matmul(
                                cps[et][:, :],
                                wout_sb[:, 2 * i + sub,
                                        et * 128:(et + 1) * 128],
                                rhs_sb[:, sub, i, :],
                                start=(sub == 0 and i == 0),
                                stop=(sub == 1 and i == NCORES - 1),
                            )
                for et in range(8):
                    ob = osb.tile([128, 512], F32, tag="osb")
                    nc.vector.tensor_copy(ob[:, :], cps[et][:, :])
                    nc.gpsimd.dma_start(out_d[et * 128:(et + 1) * 128, :],
                                        ob[:, :])

    nc.compile()
    return nc


_NC_CACHE = None


def _get_nc():
    global _NC_CACHE
    if _NC_CACHE is None:
        _NC_CACHE = build_nc()
    return _NC_CACHE


def kernel(q_x, kv_x, mask, Wq, Wkv, Wout, **_ignored):
    del mask  # all-ones by construction
    q_x = np.asarray(q_x, dtype=np.float32)
    kv_x = np.asarray(kv_x, dtype=np.float32)
    Wq = np.asarray(Wq, dtype=np.float32)
    Wkv = np.asarray(Wkv, dtype=np.float32)
    Wout = np.asarray(Wout, dtype=np.float32)

    cosq, sinq = _rope_tables(NQ)
    cosk, sink = _rope_tables(NKV)
    cosq = cosq.astype(BF16_NP)
    sinq = sinq.astype(BF16_NP)
    cosk = cosk.astype(BF16_NP)
    sink = sink.astype(BF16_NP)

    # rotate-half permutation: perm[k, d]=1 iff d=(k+32)%64 within each 64-block
    perm_np = np.zeros((128, 128), dtype=BF16_NP)
    for k in range(128):
        blk = (k // 64) * 64
        perm_np[k, blk + ((k - blk) + 32) % 64] = 1.0

    qxT = {b: np.ascontiguousarray(q_x[b].T).astype(BF16_NP) for b in range(B)}
    kvxT = {b: np.ascontiguousarray(kv_x[b].T).astype(BF16_NP) for b in range(B)}

    in_maps = []
    for c in range(NCORES):
        bi, r = c // 4, c % 4
        sl = slice(r * GD, (r + 1) * GD)
        wq_c = np.ascontiguousarray(Wq[:, sl]).astype(BF16_NP)
        wk_c = np.ascontiguousarray(Wkv[:, sl]).astype(BF16_NP)
        wv_c = np.ascontiguousarray(Wkv[:, DIM:][:, sl]).astype(BF16_NP)
        # wout shard: slot i holds Wout rows for core i's head block, zeroed
        # when core i belongs to the other batch (kills cross-batch A2A data)
        wout_c = np.zeros((NCORES, GD, DIM), dtype=BF16_NP)
        for i in range(NCORES):
            if i // 4 == bi:
                ri = i % 4
                wout_c[i] = Wout[ri * GD:(ri + 1) * GD, :].astype(BF16_NP)
        in_maps.append({
            "q_xT": qxT[bi],
            "kv_xT": kvxT[bi],
            "wq": wq_c,
            "wk": wk_c,
            "wv": wv_c,
            "wout": wout_c,
            "cosq": cosq,
            "sinq": sinq,
            "cosk": cosk,
            "sink": sink,
            "perm": perm_np,
        })

    nc = _get_nc()
    res = run_bass_kernel_spmd(nc, in_maps, core_ids=list(range(NCORES)))
    results = res.results if hasattr(res, "results") else res

    out = np.empty((B, NQ, DIM), dtype=np.float32)
    for c in range(NCORES):
        bi, r = c // 4, c % 4
        out_c = np.asarray(results[c]["out"], dtype=np.float32)  # [DIM, QS]
        out[bi, r * QS:(r + 1) * QS, :] = out_c.T
    return out


if __name__ == "__main__":
    rng = np.random.default_rng(0)
    inputs = {
        "q_x": rng.standard_normal((B, NQ, DIM), dtype=np.float32),
        "kv_x": rng.standard_normal((B, NKV, DIM), dtype=np.float32),
        "mask": np.ones((B, NKV), dtype=bool),
        "Wq": rng.standard_normal((DIM, DIM), dtype=np.float32) * 0.03,
        "Wkv": rng.standard_normal((DIM, 2 * DIM), dtype=np.float32) * 0.03,
        "Wout": rng.standard_normal((DIM, DIM), dtype=np.float32) * 0.03,
    }
    o = kernel(**inputs)
    print("kernel output", o.shape, o.dtype)
